# revision 1
# baseline (speedup 1.0000x reference)
"""Trainium2 Bass kernel for fused MultiHeadAttention + residual + LayerNorm.

Problem: B=2, L=S=2048, D=768, H=12 heads of dim 64, attention with key-padding
mask, output projection + bias, residual with q, LayerNorm(gamma, beta).

Sharding over 8 NeuronCores: data-parallel over batch (2 groups of 4 cores) x
tensor-parallel over heads (3 heads per core). Each core:
  1. projects its 3 heads' Q^T/K^T (feature-major) and V (seq-major) with f32r
     matmuls,
  2. computes S^T = K^T.T @ Q^T per head (heads 0/1 PE-row-group paired),
     exp via ScalarE with the key-padding mask folded into the activation bias,
     and O^T = [V|1].T @ P^T accumulated over s-chunks (the extra ones column
     yields the softmax denominator for free),
  3. normalizes O^T rows by the broadcast reciprocal denominator, applies the
     output projection for its 192 feature columns,
  4. ReduceScatters the partial projection over its 4-core batch group, then
     does bias + residual + LayerNorm on its 512-row shard.
Host reassembles the 8 x [512, 768] shards into (2, 2048, 768).
"""

import sys

sys.path.insert(0, "/opt/trn_rl_repo")

import ml_dtypes
import numpy as np

import concourse.bass as bass
import concourse.tile as tile
from concourse import bacc, mybir
from concourse.bass_utils import run_bass_kernel_spmd

F32 = mybir.dt.float32
F32R = mybir.dt.float32r
BF16 = mybir.dt.bfloat16
I32 = mybir.dt.int32

D = 768
HD = 64
HPC = 3  # heads per core
HCOLS = HPC * HD  # 192
B, L, S = 2, 2048, 2048
ROWS = 512  # output rows per core after ReduceScatter
NCORES = 8
GROUPS = [[0, 1, 2, 3], [4, 5, 6, 7]]
KCH = D // 128  # 6 contraction chunks for projections
SCH = S // 128  # 16 s-chunks
LBW = 512  # l-block width
LBN = L // LBW  # 4
LN_EPS = 1e-5
MASK_NEG = -1000000.0

_CACHE: dict = {}


def _build():
    nc = bacc.Bacc("TRN2", target_bir_lowering=False, debug=False, num_devices=NCORES)

    qT = nc.dram_tensor("qT", [D, L], F32R, kind="ExternalInput").ap()
    kT = nc.dram_tensor("kT", [D, S], F32R, kind="ExternalInput").ap()
    vT = nc.dram_tensor("vT", [D, S], F32R, kind="ExternalInput").ap()
    wqT = nc.dram_tensor("wqT", [D, HCOLS], F32R, kind="ExternalInput").ap()
    wkT = nc.dram_tensor("wkT", [D, HCOLS], F32R, kind="ExternalInput").ap()
    wvT = nc.dram_tensor("wvT", [D, 256], F32R, kind="ExternalInput").ap()
    wTh = [
        nc.dram_tensor(f"wTh{h}", [128, D], F32R, kind="ExternalInput").ap()
        for h in range(HPC)
    ]
    qrows = nc.dram_tensor("qrows", [LBN, 128, D], F32, kind="ExternalInput").ap()
    maskT = nc.dram_tensor("maskT", [128, SCH], I32, kind="ExternalInput").ap()
    bias1 = nc.dram_tensor("bias1", [1, D], F32, kind="ExternalInput").ap()
    gamma1 = nc.dram_tensor("gamma1", [1, D], F32, kind="ExternalInput").ap()
    beta1 = nc.dram_tensor("beta1", [1, D], F32, kind="ExternalInput").ap()
    out = nc.dram_tensor("out", [LBN, 128, D], F32, kind="ExternalOutput").ap()

    AL = mybir.AluOpType
    ACT = mybir.ActivationFunctionType

    with tile.TileContext(nc, num_cores=NCORES) as tc:
        with (
            tc.tile_pool(name="persist", bufs=1) as pp,
            tc.tile_pool(name="dram", bufs=1, space="DRAM") as dram,
        ):
            # persistent SBUF state; head-C rows are duplicated to partitions
            # 64:128 of QT2/KT2 so its S^T matmuls can PE-row-group pair.
            QT1 = pp.tile([128, L], F32R)
            QT2 = pp.tile([128, L], F32R)
            KT1 = pp.tile([128, S], F32R)
            KT2 = pp.tile([128, S], F32R)
            V_sb = pp.tile([128, SCH, HPC, 65], F32R)
            OTn = [pp.tile([128, L], F32R, name=f"OTn{h}") for h in range(HPC)]
            wq_sb = pp.tile([128, KCH, HCOLS], F32R)
            wk_sb = pp.tile([128, KCH, HCOLS], F32R)
            wv_sb = pp.tile([128, KCH, 256], F32R)
            wTh_sb = [pp.tile([128, D], F32R, name=f"wTh_sb{h}") for h in range(HPC)]
            mask_i = pp.tile([128, SCH], I32)
            mask_f = pp.tile([128, SCH], F32)
            mask_bias = pp.tile([128, SCH], F32)
            gam_b = pp.tile([128, D], F32)
            bet_b = pp.tile([128, D], F32)
            bb_b = pp.tile([128, D], F32)
            eps_t = pp.tile([128, 1], F32)

            Z_dram = dram.tile([L, D], F32)
            Zr_dram = dram.tile([LBN, 128, D], F32)

            # constant / weight loads
            nc.sync.dma_start(out=wq_sb, in_=wqT.rearrange("(c p) m -> p c m", p=128))
            nc.sync.dma_start(out=wk_sb, in_=wkT.rearrange("(c p) m -> p c m", p=128))
            nc.sync.dma_start(out=wv_sb, in_=wvT.rearrange("(c p) m -> p c m", p=128))
            for h in range(HPC):
                nc.sync.dma_start(out=wTh_sb[h], in_=wTh[h][:, :])
            nc.sync.dma_start(out=mask_i, in_=maskT[:, :])
            nc.sync.dma_start(out=gam_b, in_=gamma1.to_broadcast([128, D]))
            nc.sync.dma_start(out=bet_b, in_=beta1.to_broadcast([128, D]))
            nc.sync.dma_start(out=bb_b, in_=bias1.to_broadcast([128, D]))
            nc.vector.memset(eps_t, LN_EPS)
            ones_t = pp.tile([128, SCH, HPC, 1], F32)
            nc.vector.memset(ones_t, 1.0)
            nc.vector.tensor_copy(V_sb[:, :, :, 64:65], ones_t)
            nc.vector.tensor_copy(mask_f, mask_i)
            # (1 - m) * MASK_NEG == m * (-MASK_NEG) + MASK_NEG
            nc.scalar.activation(
                mask_bias, mask_f, ACT.Copy, bias=float(MASK_NEG), scale=-MASK_NEG
            )

            # PE warm-up: ~40 dependency-free matmuls run during the initial
            # DMA window and push the PE HAM clock-gate to 2.4 GHz before the
            # real work (f32r streams at 2 cyc/row on a cold PE, 1 warm).
            warm_f = pp.tile([128, 512], F32)
            nc.vector.memset(warm_f, 0.0)
            warm_l = pp.tile([128, 128], F32R)
            warm_r = pp.tile([128, 512], F32R)
            nc.vector.tensor_copy(warm_l, warm_f[:, 0:128])
            nc.vector.tensor_copy(warm_r, warm_f)
            with tc.tile_pool(name="warmps", bufs=1, space="PSUM") as wps:
                for w in range(40):
                    wp = wps.tile([128, 512], F32, tag="w", bufs=2, name=f"w{w}")
                    nc.tensor.matmul(wp, warm_l, warm_r, start=True, stop=True)

            # ---- Stage A: projections ----
            for xin, wsb, d1, d2 in ((qT, wq_sb, QT1, QT2), (kT, wk_sb, KT1, KT2)):
                with (
                    tc.tile_pool(name="pin", bufs=1) as pin,
                    tc.tile_pool(name="psp", bufs=1, space="PSUM") as psp,
                ):
                    chunks = []
                    for i in range(KCH):
                        ch = pin.tile([128, L], F32R, tag="in", bufs=KCH, name=f"ch{i}")
                        nc.sync.dma_start(out=ch, in_=xin[128 * i : 128 * (i + 1), :])
                        chunks.append(ch)
                    for m, mp in ((0, 128), (1, 64)):
                        for n in range(LBN):
                            ps = psp.tile([128, 512], F32, tag="ps", bufs=3, name="ps")
                            nsl = slice(512 * n, 512 * (n + 1))
                            for i in range(KCH):
                                nc.tensor.matmul(
                                    ps[:mp],
                                    wsb[:, i, 128 * m : 128 * m + mp],
                                    chunks[i][:, nsl],
                                    start=(i == 0),
                                    stop=(i == KCH - 1),
                                )
                            dest = d1 if m == 0 else d2
                            nc.any.tensor_copy(out=dest[:mp, nsl], in_=ps[:mp])
                            if m == 1:
                                # duplicate head-C rows into partitions 64:128
                                nc.sync.dma_start(
                                    out=dest[64:128, nsl], in_=dest[0:64, nsl]
                                )

            with (
                tc.tile_pool(name="pinv", bufs=1) as pin,
                tc.tile_pool(name="pspv", bufs=1, space="PSUM") as psp,
            ):
                chunks = []
                for i in range(KCH):
                    ch = pin.tile([128, S], F32R, tag="in", bufs=KCH, name=f"vch{i}")
                    nc.sync.dma_start(out=ch, in_=vT[128 * i : 128 * (i + 1), :])
                    chunks.append(ch)
                for s in range(SCH):
                    ps = psp.tile([128, 256], F32, tag="ps", bufs=3, name="psv")
                    for i in range(KCH):
                        nc.tensor.matmul(
                            ps,
                            chunks[i][:, 128 * s : 128 * (s + 1)],
                            wv_sb[:, i, :],
                            start=(i == 0),
                            stop=(i == KCH - 1),
                        )
                    nc.any.tensor_copy(
                        out=V_sb[:, s, :, 0:64],
                        in_=ps[:, 0:HCOLS].rearrange("p (h d) -> p h d", h=HPC),
                    )
                # dense keep-warm bridge: spans the pool-transition idle gap so
                # the PE clock-gate stays at 2.4 GHz entering attention
                for w in range(24):
                    wp = psp.tile([128, 256], F32, tag="ps", bufs=3, name=f"wb{w}")
                    nc.tensor.matmul(wp, warm_l, warm_r[:, 0:256], start=True, stop=True)

            # ---- Stage B+C: attention, out-projection, ReduceScatter ----
            # Per 1024-wide l-superblock: heads 0/1 (PE row-groups 0/64) write
            # one [128,1024] S^T psum covering both -> ONE exp for the pair;
            # head 2 fills the two 512 halves of the same l-superblock via its
            # duplicated Q/K rows (also row-group paired, same mask bias).
            # O^T accumulates [V|1] over s-chunks (ones col = softmax denom).
            # Then normalize, duplicate rows, paired Z matmuls, ReduceScatter.
            zmm = [0]  # running parity for Z row-group pairing

            def norm_drain(h, oH, lsl, drp):
                lnm = f"{h}_{lsl.start}"
                dr = drp.tile([65, 512], F32, tag="dr", bufs=3, name=f"dr{lnm}")
                nc.vector.reciprocal(dr[64:65, :], oH[64:65])
                nc.sync.dma_start(out=dr[0:1, :], in_=dr[64:65, :])
                rb = drp.tile([64, 512], F32, tag="rb", bufs=3, name=f"rb{lnm}")
                nc.gpsimd.partition_broadcast(rb, dr[0:1, :])
                nc.vector.tensor_mul(OTn[h][0:64, lsl], oH[0:64], rb)
                nc.sync.dma_start(out=OTn[h][64:128, lsl], in_=OTn[h][0:64, lsl])

            with (
                tc.tile_pool(name="ptp", bufs=1) as ptp,
                tc.tile_pool(name="drp", bufs=1) as drp,
                tc.tile_pool(name="zsb", bufs=3) as zsb,
                tc.tile_pool(name="aps", bufs=1, space="PSUM") as aps,
            ):
                for sb2 in range(2):  # 1024-wide l superblocks
                    l0 = 1024 * sb2
                    # heads 0+1, two 512 l-blocks
                    for half in range(2):
                        lsl = slice(l0 + 512 * half, l0 + 512 * (half + 1))
                        oA = aps.tile([128, 512], F32, tag="oA", bufs=2, name="oA")
                        oB = aps.tile([128, 512], F32, tag="oB", bufs=2, name="oB")
                        for sc in range(SCH):
                            ssl = slice(128 * sc, 128 * (sc + 1))
                            sA = aps.tile([128, 512], F32, tag="sA", bufs=2, name="sA")
                            sB = aps.tile([128, 512], F32, tag="sB", bufs=2, name="sB")
                            nc.tensor.matmul(
                                sA, KT1[0:64, ssl], QT1[0:64, lsl], start=True, stop=True
                            )
                            nc.tensor.matmul(
                                sB,
                                KT1[64:128, ssl],
                                QT1[64:128, lsl],
                                start=True,
                                stop=True,
                            )
                            pA = ptp.tile([128, 512], F32R, tag="pA", bufs=3, name="pA")
                            pB = ptp.tile([128, 512], F32R, tag="pB", bufs=3, name="pB")
                            nc.scalar.activation(
                                pA, sA, ACT.Exp, bias=mask_bias[:, sc : sc + 1], scale=0.125
                            )
                            nc.scalar.activation(
                                pB, sB, ACT.Exp, bias=mask_bias[:, sc : sc + 1], scale=0.125
                            )
                            nc.tensor.matmul(
                                oA[0:65],
                                V_sb[:, sc, 0, :],
                                pA,
                                start=(sc == 0),
                                stop=(sc == SCH - 1),
                            )
                            nc.tensor.matmul(
                                oB[0:65],
                                V_sb[:, sc, 1, :],
                                pB,
                                start=(sc == 0),
                                stop=(sc == SCH - 1),
                            )
                        norm_drain(0, oA, lsl, drp)
                        norm_drain(1, oB, lsl, drp)
                    # head 2: both 512 halves of the superblock in one pass
                    lslA = slice(l0, l0 + 512)
                    lslB = slice(l0 + 512, l0 + 1024)
                    oA = aps.tile([128, 512], F32, tag="oA", bufs=2, name="oC1")
                    oB = aps.tile([128, 512], F32, tag="oB", bufs=2, name="oC2")
                    for sc in range(SCH):
                        ssl = slice(128 * sc, 128 * (sc + 1))
                        sA = aps.tile([128, 512], F32, tag="sA", bufs=2, name="sC1")
                        sB = aps.tile([128, 512], F32, tag="sB", bufs=2, name="sC2")
                        nc.tensor.matmul(
                            sA, KT2[0:64, ssl], QT2[0:64, lslA], start=True, stop=True
                        )
                        nc.tensor.matmul(
                            sB, KT2[64:128, ssl], QT2[64:128, lslB], start=True, stop=True
                        )
                        pA = ptp.tile([128, 512], F32R, tag="pA", bufs=3, name="pC1")
                        pB = ptp.tile([128, 512], F32R, tag="pB", bufs=3, name="pC2")
                        nc.scalar.activation(
                            pA, sA, ACT.Exp, bias=mask_bias[:, sc : sc + 1], scale=0.125
                        )
                        nc.scalar.activation(
                            pB, sB, ACT.Exp, bias=mask_bias[:, sc : sc + 1], scale=0.125
                        )
                        nc.tensor.matmul(
                            oA[0:65],
                            V_sb[:, sc, 2, :],
                            pA,
                            start=(sc == 0),
                            stop=(sc == SCH - 1),
                        )
                        nc.tensor.matmul(
                            oB[0:65],
                            V_sb[:, sc, 2, :],
                            pB,
                            start=(sc == 0),
                            stop=(sc == SCH - 1),
                        )
                    norm_drain(2, oA, lslA, drp)
                    norm_drain(2, oB, lslB, drp)

                    # out-projection for this superblock (8 l-tiles), Z matmuls
                    # row-group paired via the duplicated OTn/wTh rows
                    for lt in range(8 * sb2, 8 * (sb2 + 1)):
                        tsl = slice(128 * lt, 128 * (lt + 1))
                        zp1 = aps.tile([128, 512], F32, tag="oA", bufs=2, name="zp1")
                        zp2 = aps.tile([128, 256], F32, tag="oB", bufs=2, name="zp2")
                        for n0, nw, zp in ((0, 512, zp1), (512, 256, zp2)):
                            nsl = slice(n0, n0 + nw)
                            for h in range(HPC):
                                nc.tensor.matmul(
                                    zp[:, 0:nw],
                                    OTn[h][0:64, tsl],
                                    wTh_sb[h][0:64, nsl],
                                    start=(h == 0),
                                    stop=(h == HPC - 1),
                                )
                        zs = zsb.tile([128, D], F32, name="zs")
                        nc.vector.tensor_copy(out=zs[:, 0:512], in_=zp1)
                        nc.vector.tensor_copy(out=zs[:, 512:768], in_=zp2)
                        nc.sync.dma_start(out=Z_dram[tsl, :], in_=zs)
                        if lt % 4 == 3:
                            j = lt // 4
                            nc.gpsimd.collective_compute(
                                "ReduceScatter",
                                AL.add,
                                replica_groups=GROUPS,
                                ins=[Z_dram[512 * j : 512 * (j + 1), :].opt()],
                                outs=[Zr_dram[j].opt()],
                            )

            # ---- Stage D: bias + residual + LayerNorm ----
            with tc.tile_pool(name="ep", bufs=2) as ep:
                for t in range(LBN):
                    zr = ep.tile([128, D], F32, name="zr")
                    qr = ep.tile([128, D], F32, name="qr")
                    nc.sync.dma_start(out=zr, in_=Zr_dram[t])
                    nc.sync.dma_start(out=qr, in_=qrows[t])
                    x = ep.tile([128, D], F32, name="x")
                    nc.vector.tensor_add(x, zr, qr)
                    nc.vector.tensor_add(x, x, bb_b)
                    stats = ep.tile([128, 3, 6], F32, name="stats")
                    for g in range(3):
                        nc.vector.bn_stats(stats[:, g, :], x[:, 256 * g : 256 * (g + 1)])
                    mv = ep.tile([128, 2], F32, name="mv")
                    nc.vector.bn_aggr(mv, stats)
                    rstd = ep.tile([128, 1], F32, name="rstd")
                    nc.scalar.activation(rstd, mv[:, 1:2], ACT.Sqrt, bias=eps_t, scale=1.0)
                    nc.vector.reciprocal(rstd, rstd)
                    t1 = ep.tile([128, D], F32, name="t1")
                    nc.vector.scalar_tensor_tensor(
                        t1, x, mv[:, 0:1], gam_b, AL.subtract, AL.mult
                    )
                    o = ep.tile([128, D], F32, name="o")
                    nc.vector.scalar_tensor_tensor(
                        o, t1, rstd, bet_b, AL.mult, AL.add
                    )
                    nc.sync.dma_start(out=out[t], in_=o)

    nc.finalize()
    return nc


def _get_nc():
    if "nc" not in _CACHE:
        _CACHE["nc"] = _build()
    return _CACHE["nc"]


def build_in_maps(inputs):
    return _build_in_maps(**inputs)


def _build_in_maps(q, k, v, attention_mask, Wq, Wk, Wv, W, b, gamma, beta):
    q = np.asarray(q, dtype=np.float32)
    k = np.asarray(k, dtype=np.float32)
    v = np.asarray(v, dtype=np.float32)
    attention_mask = np.asarray(attention_mask, dtype=np.int32)
    Wq = np.asarray(Wq, dtype=np.float32)
    Wk = np.asarray(Wk, dtype=np.float32)
    Wv = np.asarray(Wv, dtype=np.float32)
    W = np.asarray(W, dtype=np.float32)
    b = np.asarray(b, dtype=np.float32)
    gamma = np.asarray(gamma, dtype=np.float32)
    beta = np.asarray(beta, dtype=np.float32)

    qT = [np.ascontiguousarray(q[i].T) for i in range(B)]
    kT = [np.ascontiguousarray(k[i].T) for i in range(B)]
    vT = [np.ascontiguousarray(v[i].T) for i in range(B)]
    maskT = [np.ascontiguousarray(attention_mask[i].reshape(SCH, 128).T) for i in range(B)]
    bias1 = np.ascontiguousarray(b.reshape(1, D))
    gamma1 = np.ascontiguousarray(gamma.reshape(1, D))
    beta1 = np.ascontiguousarray(beta.reshape(1, D))

    in_maps = []
    for c in range(NCORES):
        bi, hg = c // 4, c % 4
        cs = slice(HCOLS * hg, HCOLS * (hg + 1))
        wvT_pad = np.zeros((D, 256), dtype=np.float32)
        wvT_pad[:, :HCOLS] = Wv[cs, :].T
        wT = np.ascontiguousarray(W[:, cs].T)
        in_maps.append(
            {
                "qT": qT[bi],
                "kT": kT[bi],
                "vT": vT[bi],
                "wqT": np.ascontiguousarray(Wq[cs, :].T),
                "wkT": np.ascontiguousarray(Wk[cs, :].T),
                "wvT": wvT_pad,
                "wTh0": np.ascontiguousarray(np.concatenate([wT[0:64], wT[0:64]])),
                "wTh1": np.ascontiguousarray(np.concatenate([wT[64:128], wT[64:128]])),
                "wTh2": np.ascontiguousarray(np.concatenate([wT[128:192], wT[128:192]])),
                "qrows": np.ascontiguousarray(
                    np.stack(
                        [
                            q[bi, 512 * j + 128 * hg : 512 * j + 128 * (hg + 1), :]
                            for j in range(LBN)
                        ]
                    )
                ),
                "maskT": maskT[bi],
                "bias1": bias1,
                "gamma1": gamma1,
                "beta1": beta1,
            }
        )
    return in_maps


def kernel(q, k, v, attention_mask, Wq, Wk, Wv, W, b, gamma, beta):
    nc = _get_nc()
    in_maps = _build_in_maps(q, k, v, attention_mask, Wq, Wk, Wv, W, b, gamma, beta)
    res = run_bass_kernel_spmd(nc, in_maps, core_ids=list(range(NCORES)))

    outp = np.empty((B, L, D), dtype=np.float32)
    for c in range(NCORES):
        bi, hg = c // 4, c % 4
        o = res.results[c]["out"]
        for j in range(LBN):
            outp[bi, 512 * j + 128 * hg : 512 * j + 128 * (hg + 1), :] = o[j]
    return outp



# revision 15
# speedup vs baseline: 1.3418x; 1.3418x over previous
"""Trainium2 Bass kernel for fused MultiHeadAttention + residual + LayerNorm.

Problem: B=2, L=S=2048, D=768, H=12 heads of dim 64, attention with key-padding
mask, output projection + bias, residual with q, LayerNorm(gamma, beta).

Sharding over 8 NeuronCores: data-parallel over batch (2 groups of 4 cores) x
tensor-parallel over heads (3 heads per core). Per core:
  1. project Q^T/K^T (feature-major, bf16) and V (seq-major) for its 3 heads,
  2. attention as two interleaved single-head pipelines, each owning 4 PSUM
     banks (s: 2, o: 2).  Per s-chunk: S^T matmuls -> one [128,1024] Exp
     ACTIVATE (mask folded into the per-partition bias) -> O^T accumulation
     with a ones column in V providing the softmax denominator,
  3. normalize O^T rows via reciprocal_approx_fast read straight from PSUM +
     gpsimd partition broadcast,
  4. exchange O^T head-slices with the 3 peer cores of the batch group via
     AllToAll (bf16), so each core ends with all 768 features for its own 512
     l-rows; output projection is then a local full-contraction matmul,
  5. bias + residual + LayerNorm on the own 512 rows.
Host reassembles the 8 x [4,128,768] shards into (2, 2048, 768).
"""

import sys

sys.path.insert(0, "/opt/trn_rl_repo")

import ml_dtypes
import numpy as np

import concourse.bass as bass
import concourse.tile as tile
from concourse import bacc, mybir
from concourse.bass_utils import run_bass_kernel_spmd

F32 = mybir.dt.float32
BF16 = mybir.dt.bfloat16
I32 = mybir.dt.int32

D = 768
HD = 64
HPC = 3  # heads per core
HCOLS = HPC * HD  # 192
B, L, S = 2, 2048, 2048
NCORES = 8
GROUPS = [[0, 1, 2, 3], [4, 5, 6, 7]]
KCH = D // 128  # 6 contraction chunks for projections
SCH = S // 128  # 16 s-chunks
LN_EPS = 1e-5
MASK_NEG = -1000000.0

_CACHE: dict = {}


def _build():
    nc = bacc.Bacc("TRN2", target_bir_lowering=False, debug=False, num_devices=NCORES)

    qT = nc.dram_tensor("qT", [D, L], BF16, kind="ExternalInput").ap()
    kT = nc.dram_tensor("kT", [D, S], BF16, kind="ExternalInput").ap()
    vT = nc.dram_tensor("vT", [D, S], BF16, kind="ExternalInput").ap()
    wqT = nc.dram_tensor("wqT", [D, HCOLS], BF16, kind="ExternalInput").ap()
    wkT = nc.dram_tensor("wkT", [D, HCOLS], BF16, kind="ExternalInput").ap()
    wvT = nc.dram_tensor("wvT", [D, HCOLS], BF16, kind="ExternalInput").ap()
    wtT = nc.dram_tensor("wtT", [2 * D, D], BF16, kind="ExternalInput").ap()
    qrows = nc.dram_tensor("qrows", [4, 128, D], F32, kind="ExternalInput").ap()
    maskT = nc.dram_tensor("maskT", [128, SCH], I32, kind="ExternalInput").ap()
    bias1 = nc.dram_tensor("bias1", [1, D], F32, kind="ExternalInput").ap()
    gamma1 = nc.dram_tensor("gamma1", [1, D], F32, kind="ExternalInput").ap()
    beta1 = nc.dram_tensor("beta1", [1, D], F32, kind="ExternalInput").ap()
    out = nc.dram_tensor("out", [4, 128, D], F32, kind="ExternalOutput").ap()

    AL = mybir.AluOpType
    ACT = mybir.ActivationFunctionType

    with tile.TileContext(nc, num_cores=NCORES) as tc:
        with (
            tc.tile_pool(name="persist", bufs=1) as pp,
            tc.tile_pool(name="dram", bufs=1, space="DRAM") as dram,
        ):
            # persistent SBUF state
            QT1 = pp.tile([128, L], BF16)  # heads 0 (rows 0:64) / 1 (64:128)
            QT2 = pp.tile([128, L], BF16)  # head 2, duplicated rows
            KT1 = pp.tile([128, S], BF16)
            KT2 = pp.tile([128, S], BF16)
            V_sb = pp.tile([128, SCH, HPC, 65], BF16)
            OTn = [pp.tile([64, L], BF16, name=f"OTn{h}") for h in range(HPC)]
            wq_sb = pp.tile([128, KCH, HCOLS], BF16)
            wk_sb = pp.tile([128, KCH, HCOLS], BF16)
            wv_sb = pp.tile([128, KCH, HCOLS], BF16)
            wt_sb = pp.tile([128, 2 * KCH, D], BF16)
            mask_i = pp.tile([128, SCH], I32)
            mask_f = pp.tile([128, SCH], F32)
            mask_bias = pp.tile([128, SCH], F32)
            gam_b = pp.tile([128, D], F32)
            bet_b = pp.tile([128, D], F32)
            bb_b = pp.tile([128, D], F32)
            eps_t = pp.tile([128, 1], F32)

            # AllToAll staging: [segment][192 features][2 j-sub-blocks][128 cols].
            # The collective runs over all 8 cores (4-core mesh AllToAll is
            # unsupported); each core duplicates its payload for group-peer g
            # into segments g and g+4 so receivers of either batch group find
            # their group-peers' data at those peers' global indices.  Wrong-
            # group segments are nulled by zero rows in the per-core W^T.
            atoa_in = [dram.tile([8, HCOLS, 2, 128], BF16, name=f"ai{s}") for s in range(2)]
            atoa_out = [dram.tile([8, HCOLS, 2, 128], BF16, name=f"ao{s}") for s in range(2)]

            # constant / weight loads
            nc.sync.dma_start(out=wq_sb, in_=wqT.rearrange("(c p) m -> p c m", p=128))
            nc.sync.dma_start(out=wk_sb, in_=wkT.rearrange("(c p) m -> p c m", p=128))
            nc.sync.dma_start(out=wv_sb, in_=wvT.rearrange("(c p) m -> p c m", p=128))
            nc.sync.dma_start(out=wt_sb, in_=wtT.rearrange("(c p) m -> p c m", p=128))
            nc.sync.dma_start(out=mask_i, in_=maskT[:, :])
            nc.sync.dma_start(out=gam_b, in_=gamma1.to_broadcast([128, D]))
            nc.sync.dma_start(out=bet_b, in_=beta1.to_broadcast([128, D]))
            nc.sync.dma_start(out=bb_b, in_=bias1.to_broadcast([128, D]))
            nc.vector.memset(eps_t, LN_EPS)
            ones_t = pp.tile([128, SCH, HPC, 1], BF16)
            nc.vector.memset(ones_t, 1.0)
            nc.vector.tensor_copy(V_sb[:, :, :, 64:65], ones_t)
            nc.vector.tensor_copy(mask_f, mask_i)
            # (1 - m) * MASK_NEG == m * (-MASK_NEG) + MASK_NEG
            nc.scalar.activation(
                mask_bias, mask_f, ACT.Copy, bias=float(MASK_NEG), scale=-MASK_NEG
            )

            # PE warm-up during the initial DMA window: pushes the PE HAM
            # clock-gate to 2.4 GHz before the real work.
            warm_f = pp.tile([128, 512], F32)
            nc.vector.memset(warm_f, 0.0)
            warm_l = pp.tile([128, 128], BF16)
            warm_r = pp.tile([128, 512], BF16)
            nc.vector.tensor_copy(warm_l, warm_f[:, 0:128])
            nc.vector.tensor_copy(warm_r, warm_f)
            with tc.tile_pool(name="warmps", bufs=1, space="PSUM") as wps:
                for w in range(40):
                    wp = wps.tile([128, 512], F32, tag="w", bufs=2, name=f"w{w}")
                    nc.tensor.matmul(wp, warm_l, warm_r, start=True, stop=True)

            # ---- Stage A: projections (bf16) ----
            # Q/K: feature-major Q^T/K^T; heads 0+1 -> QT1/KT1 [128, L];
            # head 2 -> QT2/KT2 rows 0:64, duplicated into 64:128.
            for xin, wsb, d1, d2 in ((kT, wk_sb, KT1, KT2), (qT, wq_sb, QT1, QT2)):
                with (
                    tc.tile_pool(name="pin", bufs=1) as pin,
                    tc.tile_pool(name="psp", bufs=1, space="PSUM") as psp,
                ):
                    chunks = []
                    for i in range(KCH):
                        ch = pin.tile([128, L], BF16, tag="in", bufs=KCH, name=f"ch{i}")
                        nc.sync.dma_start(out=ch, in_=xin[128 * i : 128 * (i + 1), :])
                        chunks.append(ch)
                    for m, mp in ((0, 128), (1, 64)):
                        for n in range(4):
                            ps = psp.tile([128, 512], F32, tag="ps", bufs=3, name="ps")
                            nsl = slice(512 * n, 512 * (n + 1))
                            for i in range(KCH):
                                nc.tensor.matmul(
                                    ps[:mp],
                                    wsb[:, i, 128 * m : 128 * m + mp],
                                    chunks[i][:, nsl],
                                    start=(i == 0),
                                    stop=(i == KCH - 1),
                                )
                            if m == 0:
                                nc.any.tensor_copy(out=d1[:, nsl], in_=ps)
                            else:
                                nc.any.tensor_copy(out=d2[0:64, nsl], in_=ps[0:64])
                                nc.any.tensor_copy(out=d2[64:128, nsl], in_=ps[0:64])

            with (
                tc.tile_pool(name="pinv", bufs=1) as pin,
                tc.tile_pool(name="pspv", bufs=1, space="PSUM") as psp,
            ):
                chunks = []
                for i in range(KCH):
                    ch = pin.tile([128, S], BF16, tag="in", bufs=KCH, name=f"vch{i}")
                    nc.sync.dma_start(out=ch, in_=vT[128 * i : 128 * (i + 1), :])
                    chunks.append(ch)
                for s in range(SCH):
                    ps = psp.tile([128, HCOLS], F32, tag="ps", bufs=3, name="psv")
                    for i in range(KCH):
                        nc.tensor.matmul(
                            ps,
                            chunks[i][:, 128 * s : 128 * (s + 1)],
                            wv_sb[:, i, :],
                            start=(i == 0),
                            stop=(i == KCH - 1),
                        )
                    nc.any.tensor_copy(
                        out=V_sb[:, s, :, 0:64],
                        in_=ps.rearrange("p (h d) -> p h d", h=HPC),
                    )

            # ---- Stage B: attention, two interleaved single-head pipelines ----
            # pass = (head, 1024-wide l-block, KT/QT row-half).  Pipelines 0/1
            # run passes 2k/2k+1 on disjoint PSUM bank halves; their S^T
            # matmuls use disjoint PE row groups so they overlap.
            passes = [
                (0, 0, QT1, KT1, 0),  # head 0, block 0, rows 0:64
                (1, 0, QT1, KT1, 64),  # head 1, block 0, rows 64:128
                (2, 0, QT2, KT2, 0),  # head 2, block 0 (dup rows 0:64)
                (2, 1, QT2, KT2, 64),  # head 2, block 1 (dup rows 64:128)
                (0, 1, QT1, KT1, 0),
                (1, 1, QT1, KT1, 64),
            ]

            def stage_ot(h, blk):
                """Stage OTn[h] block blk into the AllToAll input buffers."""
                otr = OTn[h].rearrange("p (j i) -> p j i", j=4)  # [64, 4, 512]
                for g in range(4):
                    # cols 512*j + 128*g + i for j in {2*blk, 2*blk+1}
                    src = otr[:, 2 * blk : 2 * blk + 2, 128 * g : 128 * (g + 1)]
                    for seg in (g, g + 4):
                        nc.gpsimd.dma_start(
                            out=atoa_in[blk][seg, 64 * h : 64 * (h + 1)], in_=src
                        )

            with (
                tc.tile_pool(name="ptp", bufs=1) as ptp,
                tc.tile_pool(name="drp", bufs=1) as drp,
                tc.tile_pool(name="aps", bufs=1, space="PSUM") as aps,
            ):
                otile = [None, None]

                def emit_st(pl, sc):
                    h, blk, QTx, KTx, r0 = passes[pl[0]]
                    l0 = 1024 * blk
                    st = aps.tile([128, 1024], F32, tag=f"s{pl[1]}", bufs=1, name=f"s{pl[1]}")
                    ssl = slice(128 * sc, 128 * (sc + 1))
                    for half in range(2):
                        nc.tensor.matmul(
                            st[:, 512 * half : 512 * (half + 1)],
                            KTx[r0 : r0 + 64, ssl],
                            QTx[r0 : r0 + 64, l0 + 512 * half : l0 + 512 * (half + 1)],
                            start=True,
                            stop=True,
                        )
                    return st

                def emit_exp(pl, sc, st):
                    p = ptp.tile([128, 1024], BF16, tag=f"p{pl[1]}", bufs=2, name=f"p{pl[1]}")
                    nc.scalar.activation(
                        p, st, ACT.Exp, bias=mask_bias[:, sc : sc + 1], scale=0.125
                    )
                    return p

                def emit_ot(pl, sc, p):
                    h = passes[pl[0]][0]
                    ot = otile[pl[1]]
                    for half in range(2):
                        nc.tensor.matmul(
                            ot[0:65, 512 * half : 512 * (half + 1)],
                            V_sb[:, sc, h, :],
                            p[:, 512 * half : 512 * (half + 1)],
                            start=(sc == 0),
                            stop=(sc == SCH - 1),
                        )

                def norm_drain(pl):
                    h, blk = passes[pl[0]][0], passes[pl[0]][1]
                    l0 = 1024 * blk
                    ot = otile[pl[1]]
                    nm = f"{pl[1]}"
                    otmp = drp.tile([65, 1024], F32, tag="ox" + nm, bufs=2, name="ox" + nm)
                    nc.vector.tensor_copy(otmp, ot[0:65, :])
                    dr = drp.tile([1, 1024], F32, tag="dr" + nm, bufs=2, name="dr" + nm)
                    nc.vector.reciprocal(dr, otmp[64:65, :])
                    rb = drp.tile([64, 1024], F32, tag="rb" + nm, bufs=2, name="rb" + nm)
                    nc.gpsimd.partition_broadcast(rb, dr)
                    nc.vector.tensor_mul(
                        OTn[h][:, l0 : l0 + 1024], otmp[0:64, :], rb
                    )
                    stage_ot(h, blk)

                for pp_i in range(3):
                    pls = [(2 * pp_i, 0), (2 * pp_i + 1, 1)]
                    for pl in pls:
                        otile[pl[1]] = aps.tile(
                            [128, 1024], F32, tag=f"o{pl[1]}", bufs=1, name=f"o{pl[1]}"
                        )
                    for sc in range(SCH):
                        new_sts = [emit_st(pl, sc) for pl in pls]
                        new_ps = [emit_exp(pl, sc, new_sts[i]) for i, pl in enumerate(pls)]
                        for i, pl in enumerate(pls):
                            emit_ot(pl, sc, new_ps[i])
                    for pl in pls:
                        norm_drain(pl)
                    if pp_i == 1:
                        # block-0 O^T slices for all heads are staged (passes
                        # 0,1,2); exchange them while block-1 attention runs.
                        nc.gpsimd.collective_compute(
                            "AllToAll",
                            AL.bypass,
                            replica_groups=[list(range(8))],
                            ins=[atoa_in[0].opt()],
                            outs=[atoa_out[0].opt()],
                        )
                nc.gpsimd.collective_compute(
                    "AllToAll",
                    AL.bypass,
                    replica_groups=[list(range(8))],
                    ins=[atoa_in[1].opt()],
                    outs=[atoa_out[1].opt()],
                )

            # ---- Stage C: local output projection + residual + LayerNorm ----
            # atoa_out[sb][g, f, jj, :] holds peer g's O^T features for own
            # l-row-block j = 2*sb + jj.  Global feature = 192*g + f; load as
            # 6 chunks of 128 partitions, matmul against W^T chunks.
            with (
                tc.tile_pool(name="zc", bufs=2) as zc,
                tc.tile_pool(name="ep", bufs=2) as ep,
                tc.tile_pool(name="zps", bufs=1, space="PSUM") as zpsp,
            ):
                for j in range(4):
                    sb, jj = j // 2, j % 2
                    zcj = zc.tile([128, 2 * KCH, 128], BF16, name=f"zcj{j}")
                    for ci in range(2 * KCH):
                        f0 = 128 * ci  # global (segment-major) feature of partition 0
                        g0, r0 = divmod(f0, HCOLS)
                        n0 = min(HCOLS - r0, 128)
                        nc.sync.dma_start(
                            out=zcj[0:n0, ci],
                            in_=atoa_out[sb][g0, r0 : r0 + n0, jj],
                        )
                        if n0 < 128:
                            nc.sync.dma_start(
                                out=zcj[n0:128, ci],
                                in_=atoa_out[sb][g0 + 1, 0 : 128 - n0, jj],
                            )
                    zp = zpsp.tile([128, D], F32, tag="zp", bufs=2, name=f"zp{j}")
                    for n0, nw in ((0, 512), (512, 256)):
                        for ci in range(2 * KCH):
                            nc.tensor.matmul(
                                zp[:, n0 : n0 + nw],
                                zcj[:, ci, :],
                                wt_sb[:, ci, n0 : n0 + nw],
                                start=(ci == 0),
                                stop=(ci == 2 * KCH - 1),
                            )
                    qr = ep.tile([128, D], F32, name="qr")
                    nc.sync.dma_start(out=qr, in_=qrows[j])
                    x = ep.tile([128, D], F32, name="x")
                    nc.vector.tensor_add(x, zp, qr)
                    nc.vector.tensor_add(x, x, bb_b)
                    stats = ep.tile([128, 3, 6], F32, name="stats")
                    for g in range(3):
                        nc.vector.bn_stats(stats[:, g, :], x[:, 256 * g : 256 * (g + 1)])
                    mv = ep.tile([128, 2], F32, name="mv")
                    nc.vector.bn_aggr(mv, stats)
                    rstd = ep.tile([128, 1], F32, name="rstd")
                    nc.scalar.activation(rstd, mv[:, 1:2], ACT.Sqrt, bias=eps_t, scale=1.0)
                    nc.vector.reciprocal(rstd, rstd)
                    t1 = ep.tile([128, D], F32, name="t1")
                    nc.vector.scalar_tensor_tensor(
                        t1, x, mv[:, 0:1], gam_b, AL.subtract, AL.mult
                    )
                    o = ep.tile([128, D], F32, name="o")
                    nc.vector.scalar_tensor_tensor(
                        o, t1, rstd, bet_b, AL.mult, AL.add
                    )
                    nc.sync.dma_start(out=out[j], in_=o)

    nc.finalize()
    return nc


def _get_nc():
    if "nc" not in _CACHE:
        _CACHE["nc"] = _build()
    return _CACHE["nc"]


def build_in_maps(inputs):
    return _build_in_maps(**inputs)


def _bf16(x):
    return np.ascontiguousarray(x.astype(ml_dtypes.bfloat16))


def _build_in_maps(q, k, v, attention_mask, Wq, Wk, Wv, W, b, gamma, beta):
    q = np.asarray(q, dtype=np.float32)
    k = np.asarray(k, dtype=np.float32)
    v = np.asarray(v, dtype=np.float32)
    attention_mask = np.asarray(attention_mask, dtype=np.int32)
    Wq = np.asarray(Wq, dtype=np.float32)
    Wk = np.asarray(Wk, dtype=np.float32)
    Wv = np.asarray(Wv, dtype=np.float32)
    W = np.asarray(W, dtype=np.float32)
    b = np.asarray(b, dtype=np.float32)
    gamma = np.asarray(gamma, dtype=np.float32)
    beta = np.asarray(beta, dtype=np.float32)

    qT = [_bf16(q[i].T) for i in range(B)]
    kT = [_bf16(k[i].T) for i in range(B)]
    vT = [_bf16(v[i].T) for i in range(B)]
    Wt = W.T  # [in-feature f, out-feature d]
    # Extended W^T over the 8 AllToAll segments: segment s carries core s's
    # head-group (s % 4) features; zero rows null the wrong-batch segments.
    wtT = []
    for bi in range(B):
        w = np.zeros((2 * D, D), dtype=np.float32)
        for s_ in range(4 * bi, 4 * bi + 4):
            hg = s_ % 4
            w[HCOLS * s_ : HCOLS * (s_ + 1), :] = Wt[HCOLS * hg : HCOLS * (hg + 1), :]
        wtT.append(_bf16(w))
    maskT = [np.ascontiguousarray(attention_mask[i].reshape(SCH, 128).T) for i in range(B)]
    bias1 = np.ascontiguousarray(b.reshape(1, D))
    gamma1 = np.ascontiguousarray(gamma.reshape(1, D))
    beta1 = np.ascontiguousarray(beta.reshape(1, D))

    in_maps = []
    for c in range(NCORES):
        bi, hg = c // 4, c % 4
        cs = slice(HCOLS * hg, HCOLS * (hg + 1))
        in_maps.append(
            {
                "qT": qT[bi],
                "kT": kT[bi],
                "vT": vT[bi],
                "wqT": _bf16(Wq[cs, :].T),
                "wkT": _bf16(Wk[cs, :].T),
                "wvT": _bf16(Wv[cs, :].T),
                "wtT": wtT[bi],
                "qrows": np.ascontiguousarray(
                    np.stack(
                        [
                            q[bi, 512 * j + 128 * hg : 512 * j + 128 * (hg + 1), :]
                            for j in range(4)
                        ]
                    )
                ),
                "maskT": maskT[bi],
                "bias1": bias1,
                "gamma1": gamma1,
                "beta1": beta1,
            }
        )
    return in_maps


def kernel(q, k, v, attention_mask, Wq, Wk, Wv, W, b, gamma, beta):
    nc = _get_nc()
    in_maps = _build_in_maps(q, k, v, attention_mask, Wq, Wk, Wv, W, b, gamma, beta)
    res = run_bass_kernel_spmd(nc, in_maps, core_ids=list(range(NCORES)))

    outp = np.empty((B, L, D), dtype=np.float32)
    for c in range(NCORES):
        bi, hg = c // 4, c % 4
        o = res.results[c]["out"]
        for j in range(4):
            outp[bi, 512 * j + 128 * hg : 512 * j + 128 * (hg + 1), :] = o[j]
    return outp
